# revision 8
# baseline (speedup 1.0000x reference)
"""GPT2 causal attention (B=2, T=2048, C=1024, H=16) on 8 TRN2 NeuronCores.

Sharding: core g = (batch b = g//4, head-group hg = g%4 of 4 heads).
Tensor-parallel over heads (column-split W_attn, row-split W_proj) x
data-parallel over batch. Each core computes a full [T, C] partial of the
output projection for its 4 heads; host sums the 4 partials per batch and
adds b_proj. No collectives.

Per-core kernel (bf16 matmuls, fp32 PSUM):
  qT/kT = (W^T x^T) in [d, T] layout, V in [T, d] natural layout with a
  ones-column appended per head (so the attention*V matmul also produces
  the softmax row-sums). Scores are computed transposed, S^T[tk, tq] =
  kT_tile^T @ qT, exp'd without max-subtraction (scores ~ N(0,1)), causal
  tiles only with a host-provided mask on the diagonal band, then
  Yu^T[d, tq] = V_aug^T @ expS^T accumulates over tk. Row sums are
  reciprocal'd via a small DMA transpose (for 128-lane DVE recip) and
  broadcast back; yT is normalized in place and fed to the output
  projection as the stationary operand.
"""

import numpy as np
import ml_dtypes

BF16 = ml_dtypes.bfloat16

B, T, C, H, D = 2, 2048, 1024, 16, 64
HL = 4          # heads per core
DL = HL * D     # 256 local head dims
N_CORES = 8
NT = T // 128   # 16 tk tiles
NJ = T // 512   # 4 tq groups
SCALE = 1.0 / np.sqrt(D)

_CACHE = {}


def _build_program():
    import concourse.tile as tile
    from concourse import bacc
    import concourse.mybir as mybir

    f32 = mybir.dt.float32
    bf16 = mybir.dt.bfloat16
    Exp = mybir.ActivationFunctionType.Exp

    nc = bacc.Bacc("TRN2", target_bir_lowering=False, debug=False)

    # ---- DRAM I/O (per-core, host pre-sharded) ----
    xT_d = nc.dram_tensor("xT", [C, T], bf16, kind="ExternalInput").ap()
    wq_d = nc.dram_tensor("wq", [C, DL], bf16, kind="ExternalInput").ap()
    wk_d = nc.dram_tensor("wk", [C, DL], bf16, kind="ExternalInput").ap()
    wv_d = nc.dram_tensor("wv", [C, DL], bf16, kind="ExternalInput").ap()
    wp_d = nc.dram_tensor("wp", [DL, C], bf16, kind="ExternalInput").ap()
    bq_d = nc.dram_tensor("bq", [128, 2], f32, kind="ExternalInput").ap()
    bk_d = nc.dram_tensor("bk", [128, 2], f32, kind="ExternalInput").ap()
    bvr_d = nc.dram_tensor("bvr", [128, DL], f32, kind="ExternalInput").ap()
    mask_d = nc.dram_tensor("masks", [128, 4 * 512], bf16, kind="ExternalInput").ap()
    out_d = nc.dram_tensor("out", [T, C], f32, kind="ExternalOutput").ap()
    s_dram = nc.dram_tensor("s_scratch", [HL * T], f32).ap()
    r_dram = nc.dram_tensor("r_scratch", [HL * T], bf16).ap()

    with tile.TileContext(nc) as tc:
        with (
            tc.tile_pool(name="const", bufs=1) as cpool,
            tc.tile_pool(name="exp", bufs=4) as epool,
            tc.tile_pool(name="rep", bufs=4) as rpool,
            tc.tile_pool(name="small", bufs=2) as spool,
            tc.tile_pool(name="ostage", bufs=4) as opool,
            tc.tile_pool(name="psbig", bufs=2, space="PSUM") as pbig,
            tc.tile_pool(name="psyu", bufs=4, space="PSUM") as pyu,
        ):
            # ---- persistent SBUF ----
            xT = cpool.tile([128, 8 * T], bf16, tag="xT")       # c-chunk c at [:, c*T:]
            wq = cpool.tile([128, 8 * DL], bf16, tag="wq")
            wk = cpool.tile([128, 8 * DL], bf16, tag="wk")
            wv = cpool.tile([128, 8 * DL], bf16, tag="wv")
            wp = cpool.tile([128, 2 * C], bf16, tag="wp")       # d-chunk dc at [:, dc*C:]
            bq = cpool.tile([128, 2], f32, tag="bq")
            bk = cpool.tile([128, 2], f32, tag="bk")
            bvr = cpool.tile([128, DL], f32, tag="bvr")
            masks = cpool.tile([128, 4 * 512], bf16, tag="masks")
            qT = cpool.tile([128, 2 * T], bf16, tag="qT")       # head h: [64*(h%2):, (h//2)*T + t]
            kT = cpool.tile([128, 2 * T], bf16, tag="kT")
            yT = cpool.tile([128, 2 * T], bf16, tag="yT")
            V = cpool.tile([128, NT * (HL * 65)], bf16, tag="V")  # t-tile tt, head h at [:, tt*260 + 65*h : +65]

            # ---- load inputs ----
            for c in range(8):
                nc.sync.dma_start(out=xT[:, c * T:(c + 1) * T], in_=xT_d[c * 128:(c + 1) * 128, :])
                nc.sync.dma_start(out=wq[:, c * DL:(c + 1) * DL], in_=wq_d[c * 128:(c + 1) * 128, :])
                nc.sync.dma_start(out=wk[:, c * DL:(c + 1) * DL], in_=wk_d[c * 128:(c + 1) * 128, :])
                nc.sync.dma_start(out=wv[:, c * DL:(c + 1) * DL], in_=wv_d[c * 128:(c + 1) * 128, :])
            for dc in range(2):
                nc.sync.dma_start(out=wp[:, dc * C:(dc + 1) * C], in_=wp_d[dc * 128:(dc + 1) * 128, :])
            nc.sync.dma_start(out=bq[:, :], in_=bq_d[:, :])
            nc.sync.dma_start(out=bk[:, :], in_=bk_d[:, :])
            nc.sync.dma_start(out=bvr[:, :], in_=bvr_d[:, :])
            nc.sync.dma_start(out=masks[:, :], in_=mask_d[:, :])

            # ---- QKV projections ----
            # qT/kT: out [d-chunk 128, t-slice 512] = W_chunk^T @ xT_chunk
            for (w_sb, b_sb, dst) in ((wq, bq, qT), (wk, bk, kT)):
                for dc in range(2):
                    for ts in range(4):
                        ps = pbig.tile([128, 512], f32, tag="big")
                        for c in range(8):
                            nc.tensor.matmul(
                                ps[:, :],
                                w_sb[:, c * DL + dc * 128: c * DL + (dc + 1) * 128],
                                xT[:, c * T + ts * 512: c * T + (ts + 1) * 512],
                                start=(c == 0), stop=(c == 7),
                            )
                        nc.vector.tensor_scalar_add(
                            dst[:, dc * T + ts * 512: dc * T + (ts + 1) * 512],
                            ps[:, :], b_sb[:, dc:dc + 1],
                        )

            # V natural [t, d]: stationary xT chunk, moving W_v chunk.
            nc.vector.memset(V[:, :], 1.0)  # ones-columns; data cols overwritten
            for tt in range(NT):
                ps = pbig.tile([128, DL], f32, tag="big")
                for c in range(8):
                    nc.tensor.matmul(
                        ps[:, :],
                        xT[:, c * T + tt * 128: c * T + (tt + 1) * 128],
                        wv[:, c * DL:(c + 1) * DL],
                        start=(c == 0), stop=(c == 7),
                    )
                vdst = V[:, tt * (HL * 65): (tt + 1) * (HL * 65)].rearrange(
                    "p (h e) -> p h e", h=HL)[:, :, 0:64]
                nc.vector.tensor_add(
                    vdst,
                    ps[:, :].rearrange("p (h e) -> p h e", h=HL),
                    bvr[:, :].rearrange("p (h e) -> p h e", h=HL),
                )

            # ---- attention per head ----
            for h in range(HL):
                hp, half = divmod(h, 2)
                po = 64 * half            # partition offset of this head in qT/kT/yT
                fb = hp * T               # free-dim base
                s_h = spool.tile([65, T], f32, tag="s_h", name=f"s_h_{h}")
                for jlist in ([0, 1], [2, 3]):
                    yu = {}
                    for j in jlist:
                        yu[j] = pyu.tile([65, 512], f32, tag="yu", name=f"yu_{h}_{j}")
                    for i in range(NT):
                        j0 = i // 4
                        js = [j for j in jlist if j >= j0]
                        if not js:
                            continue
                        jr0 = js[0] - jlist[0]
                        sc = pbig.tile([128, 1024], f32, tag="big")
                        for j in js:
                            jr = j - jlist[0]
                            nc.tensor.matmul(
                                sc[:, jr * 512:(jr + 1) * 512],
                                kT[po:po + 64, fb + i * 128: fb + (i + 1) * 128],
                                qT[po:po + 64, fb + j * 512: fb + (j + 1) * 512],
                                start=True, stop=True,
                            )
                        et = epool.tile([128, 1024], bf16, tag="exp")
                        nc.scalar.activation(
                            et[:, jr0 * 512: 1024], sc[:, jr0 * 512: 1024],
                            Exp, scale=float(SCALE),
                        )
                        if j0 in jlist:
                            v = i % 4
                            w = (v + 1) * 128
                            jd = (j0 - jlist[0]) * 512
                            nc.vector.tensor_mul(
                                et[:, jd: jd + w],
                                et[:, jd: jd + w],
                                masks[:, v * 512: v * 512 + w],
                            )
                        for j in js:
                            jr = j - jlist[0]
                            nc.tensor.matmul(
                                yu[j][:, :],
                                V[:, i * (HL * 65) + 65 * h: i * (HL * 65) + 65 * h + 65],
                                et[:, jr * 512:(jr + 1) * 512],
                                start=(i == 0), stop=(i == 4 * j + 3),
                            )
                    # DVE lanes are physical: PSUM partition p can only be
                    # copied to SBUF partition p. Odd heads (po=64) bounce
                    # through a partition-0 stage + DMA to cross partitions.
                    for j in jlist:
                        if po == 0:
                            nc.vector.tensor_copy(
                                yT[0:64, fb + j * 512: fb + (j + 1) * 512],
                                yu[j][0:64, :],
                            )
                        else:
                            yst = spool.tile([64, 512], bf16, tag="ystage",
                                             name=f"yst_{h}_{j}")
                            nc.vector.tensor_copy(yst[:, :], yu[j][0:64, :])
                            nc.sync.dma_start(
                                out=yT[64:128, fb + j * 512: fb + (j + 1) * 512],
                                in_=yst[:, :],
                            )
                        nc.vector.tensor_copy(
                            s_h[64:65, j * 512:(j + 1) * 512],
                            yu[j][64:65, :],
                        )

                # reciprocal of row sums: bounce through DRAM to transpose
                # [1, T] into [128, 16] so DVE recip runs on 128 lanes.
                nc.sync.dma_start(
                    out=s_dram[h * T:(h + 1) * T],
                    in_=s_h[64:65, :],
                )
                sT = spool.tile([128, 16], f32, tag="sT")
                nc.sync.dma_start(
                    out=sT[:, :],
                    in_=s_dram[h * T:(h + 1) * T].rearrange("(c p) -> p c", p=128),
                )
                rT = spool.tile([128, 16], f32, tag="rT")
                nc.vector.reciprocal(rT[:, :], sT[:, :])
                rTb = spool.tile([128, 16], bf16, tag="rTb")
                nc.vector.tensor_copy(rTb[:, :], rT[:, :])
                nc.sync.dma_start(
                    out=r_dram[h * T:(h + 1) * T].rearrange("(c p) -> p c", p=128),
                    in_=rTb[:, :],
                )
                for j in range(NJ):
                    rep = rpool.tile([128, 512], bf16, tag="rep")
                    nc.sync.dma_start(
                        out=rep[po:po + 64, :],
                        in_=r_dram[h * T + j * 512: h * T + (j + 1) * 512].partition_broadcast(64),
                    )
                    nc.vector.tensor_mul(
                        yT[po:po + 64, fb + j * 512: fb + (j + 1) * 512],
                        yT[po:po + 64, fb + j * 512: fb + (j + 1) * 512],
                        rep[po:po + 64, :],
                    )

            # ---- output projection: out[t, c] = sum_d yT[d, t] * wp[d, c] ----
            for tt in range(NT):
                for cc in range(2):
                    pp = pbig.tile([128, 512], f32, tag="big")
                    for dc in range(2):
                        nc.tensor.matmul(
                            pp[:, :],
                            yT[:, dc * T + tt * 128: dc * T + (tt + 1) * 128],
                            wp[:, dc * C + cc * 512: dc * C + (cc + 1) * 512],
                            start=(dc == 0), stop=(dc == 1),
                        )
                    ot = opool.tile([128, 512], f32, tag="ot")
                    if (tt + cc) % 2 == 0:
                        nc.scalar.copy(ot[:, :], pp[:, :])
                    else:
                        nc.vector.tensor_copy(ot[:, :], pp[:, :])
                    nc.sync.dma_start(
                        out=out_d[tt * 128:(tt + 1) * 128, cc * 512:(cc + 1) * 512],
                        in_=ot[:, :],
                    )

    nc.compile()
    return nc


def get_program():
    if "nc" not in _CACHE:
        _CACHE["nc"] = _build_program()
    return _CACHE["nc"]


def make_in_maps(x, W_attn, b_attn, W_proj):
    """Host-side sharding: per-core input dict."""
    x = np.asarray(x, np.float32)
    W_attn = np.asarray(W_attn, np.float32)
    b_attn = np.asarray(b_attn, np.float32)
    W_proj = np.asarray(W_proj, np.float32)

    tk = np.arange(128)[:, None]
    tq = np.arange(512)[None, :]
    masks = np.zeros((128, 4 * 512), BF16)
    for v in range(4):
        masks[:, v * 512:(v + 1) * 512] = (tq >= 128 * v + tk).astype(BF16)

    xT_b = [np.ascontiguousarray(x[b].T).astype(BF16) for b in range(B)]

    in_maps = []
    for g in range(N_CORES):
        b, hg = divmod(g, 4)
        cs = slice(hg * DL, (hg + 1) * DL)
        wq = np.ascontiguousarray(W_attn[:, 0 * C:1 * C][:, cs]).astype(BF16)
        wk = np.ascontiguousarray(W_attn[:, 1 * C:2 * C][:, cs]).astype(BF16)
        wv = np.ascontiguousarray(W_attn[:, 2 * C:3 * C][:, cs]).astype(BF16)
        wp = np.ascontiguousarray(W_proj[cs, :]).astype(BF16)
        bq = np.ascontiguousarray(b_attn[0 * C:1 * C][cs].reshape(2, 128).T)
        bk = np.ascontiguousarray(b_attn[1 * C:2 * C][cs].reshape(2, 128).T)
        bvr = np.ascontiguousarray(np.tile(b_attn[2 * C:3 * C][cs][None, :], (128, 1)))
        in_maps.append({
            "xT": xT_b[b],
            "wq": wq, "wk": wk, "wv": wv, "wp": wp,
            "bq": bq.astype(np.float32), "bk": bk.astype(np.float32),
            "bvr": bvr.astype(np.float32),
            "masks": masks,
        })
    return in_maps


def assemble_output(results, b_proj):
    """results: per-core dicts with 'out' [T, C] partials."""
    b_proj = np.asarray(b_proj, np.float32)
    out = np.zeros((B, T, C), np.float32)
    for g in range(N_CORES):
        out[g // 4] += np.asarray(results[g]["out"], np.float32)
    out += b_proj[None, None, :]
    return out


def kernel(x, W_attn, b_attn, W_proj, b_proj):
    from concourse.bass_utils import run_bass_kernel_spmd

    nc = get_program()
    in_maps = make_in_maps(x, W_attn, b_attn, W_proj)
    res = run_bass_kernel_spmd(nc, in_maps, list(range(N_CORES)))
    return assemble_output(res.results, b_proj)
